# revision 9
# baseline (speedup 1.0000x reference)
"""Causal self-attention (B=4, S=2048, D=1024, H=16, hd=64) on 8 TRN2 NeuronCores.

Sharding: batch 4-way x head-group 2-way. Core c = 2*b + g handles batch b and
heads [8g, 8g+8). Each core computes the QKV projection for its heads, causal
flash-style attention, and a partial output projection; the host sums the two
head-group partials per batch.

v2 schedule: the PE is kept continuously busy (p-state!) by a global deque of
~1.7us "filler" units (projection half-pieces, deferred out-projection units)
popped between attention chunk-pairs to cover exp() latency:
  - stripe 3's q-projection is pulled two phases early so sb3's attention can
    start the moment k chunks land;
  - every superblock's out-projection is deferred into the next superblock;
  - a few fillers are held back to keep the PE warm through the last
    normalize, so the final out-projection runs at max clock;
  - y PSUM->SBUF copies run on the ACT engine, triangle masks on GpSimd,
    keeping DVE off the critical path.
Attention per (head-pair, chunk-pair): S^T = K.Q^T lands P^T via exp() in the
layout the P^T.V matmul wants; a ones-column in V yields softmax denominators
(row 64). Head pairs sit on disjoint PE row halves so their QK matmuls stream
concurrently. No running-max: scores are bounded, exp stays finite in fp32.
"""

import sys

for _p in ("/opt/trn_rl_repo",):
    if _p not in sys.path:
        sys.path.insert(0, _p)

from contextlib import ExitStack

import numpy as np

import concourse.bass as bass
import concourse.mybir as mybir
import concourse.tile as tile
from concourse import bacc
from concourse.bass_utils import run_bass_kernel_spmd

F32 = mybir.dt.float32
BF16 = mybir.dt.bfloat16
P = 128
B, S, D = 4, 2048, 1024
HD = 64          # head dim
NH = 8           # heads per core
KO = D // P      # 8 contraction chunks for the projections
QSB = 512        # q superblock (matmul free dim)
N_SB = S // QSB  # 4
N_SC = S // P    # 16 kv chunks
PSTRIPE = 512    # s-stripe for the projection phase
SCALE = 0.125    # 1/sqrt(64)


def _attention_kernel(tc, out, xT, w_qk, w_v, w_out):
    nc = tc.nc
    with ExitStack() as ctx:
        const_pool = ctx.enter_context(tc.tile_pool(name="const", bufs=1))
        qkT_pool = ctx.enter_context(tc.tile_pool(name="qkT", bufs=1))
        v_pool = ctx.enter_context(tc.tile_pool(name="vsb", bufs=1))
        wqk_pool = ctx.enter_context(tc.tile_pool(name="wqk", bufs=1))
        wv_pool = ctx.enter_context(tc.tile_pool(name="wv", bufs=1))
        wout_pool = ctx.enter_context(tc.tile_pool(name="wout", bufs=1))
        xt_pool = ctx.enter_context(tc.tile_pool(name="xt", bufs=3))
        pt_pool = ctx.enter_context(tc.tile_pool(name="pt", bufs=6))
        y_pool = ctx.enter_context(tc.tile_pool(name="ysb", bufs=2))
        r_pool = ctx.enter_context(tc.tile_pool(name="recip", bufs=6))
        o_pool = ctx.enter_context(tc.tile_pool(name="osb", bufs=3))
        # PSUM: scores 2x2 banks + work(proj/out) 2x1 + y accum 2x1 = 8 banks
        ps_sc = ctx.enter_context(tc.tile_pool(name="ps_sc", bufs=2, space="PSUM"))
        ps_wk = ctx.enter_context(tc.tile_pool(name="ps_wk", bufs=2, space="PSUM"))
        ps_y = ctx.enter_context(tc.tile_pool(name="ps_y", bufs=2, space="PSUM"))

        # ones row for the 1/den broadcast outer product (bf16: PE streams
        # 1 col/cycle vs 4 for f32)
        ones64 = const_pool.tile([1, HD], BF16, tag="ones64")
        nc.gpsimd.memset(ones64[:], 1.0)

        # p-state warmup: junk matmuls with no DMA dependencies keep the
        # PE busy (and its clock ramping) while the first inputs land.
        warm_w = const_pool.tile([P, P], BF16, tag="warmw")
        nc.gpsimd.memset(warm_w[:], 1.0)
        warm_x = const_pool.tile([P, 512], BF16, tag="warmx")
        nc.gpsimd.memset(warm_x[:], 1.0)
        warm_ps = ps_y.tile([P, 512], F32, tag="ps_y", name="warmps")
        # q^T/k^T store: row-chunk rc<4 holds q rows, rc>=4 holds k rows.
        # Head h lives at partitions 64*(h%2)..+64 of row-chunk h//2 (+4 for k).
        qkT = qkT_pool.tile([P, 8, S], BF16)
        for _w in range(16):
            nc.tensor.matmul(warm_ps[:], lhsT=warm_w[:], rhs=warm_x[:],
                             start=True, stop=True)
        # V store: [s-partition, kv-chunk, head, hd+1]; last col is ones for the
        # softmax denominator.
        v_sb = v_pool.tile([P, N_SC, NH, HD + 1], BF16)
        nc.gpsimd.memset(v_sb[:, :, :, HD], 1.0)

        # stripe-0 x chunks interleave with the weight chunks so the first
        # projection matmul starts after ~2 chunks instead of the full 5 MB
        wqk_sb = wqk_pool.tile([P, KO, 2 * 512], BF16)
        xts = [None] * 4
        xts[0] = xt_pool.tile([P, KO, PSTRIPE], BF16, tag="xt", name="xt0")
        for ko in range(KO):
            for cg in range(2):
                nc.sync.dma_start(
                    wqk_sb[:, ko, cg * 512:(cg + 1) * 512],
                    w_qk[ko * P:(ko + 1) * P, cg * 512:(cg + 1) * 512],
                )
            nc.sync.dma_start(xts[0][:, ko, :], xT[ko * P:(ko + 1) * P, 0:PSTRIPE])
        wv_sb = wv_pool.tile([P, KO, 512], BF16)
        for ko in range(KO):
            nc.sync.dma_start(wv_sb[:, ko, :], w_v[ko * P:(ko + 1) * P, :])
        wout_sb = wout_pool.tile([P, 4, D], BF16)
        for co in range(4):
            nc.sync.dma_start(wout_sb[:, co, :], w_out[co * P:(co + 1) * P, :])

        def load_stripe(st):
            xt = xt_pool.tile([P, KO, PSTRIPE], BF16, tag="xt", name=f"xt{st}")
            for ko in range(KO):
                nc.sync.dma_start(
                    xt[:, ko, :],
                    xT[ko * P:(ko + 1) * P, st * PSTRIPE:(st + 1) * PSTRIPE],
                )
            xts[st] = xt

        # ---- filler units: closures of ~0.9-1.9us of PE work ----

        def qk_piece(st, rcp):
            """q^T/k^T rows for row-chunks 2*rcp..2*rcp+1 of stripe st.
            Two self-contained closures (one 1-bank psum tile per row-chunk)."""

            def one(rc, alt=False):
                pool, tg = (ps_y, "ps_y") if alt else (ps_wk, "ps_wk")
                ps = pool.tile([P, PSTRIPE], F32, tag=tg, name=f"pqk{st}_{rc}")
                for ko in range(KO):
                    nc.tensor.matmul(
                        ps[:],
                        lhsT=wqk_sb[:, ko, rc * P:(rc + 1) * P],
                        rhs=xts[st][:, ko, :],
                        start=(ko == 0),
                        stop=(ko == KO - 1),
                    )
                nc.vector.tensor_copy(
                    qkT[:, rc, st * PSTRIPE:(st + 1) * PSTRIPE], ps[:]
                )

            return [lambda: one(2 * rcp, st == 0),
                    lambda: one(2 * rcp + 1, False)]

        def v_piece(st, subp):
            """V rows for s-chunks 4*st+2*subp..+1. Two self-contained closures."""

            def one(sub, alt=False):
                pool, tg = (ps_y, "ps_y") if alt else (ps_wk, "ps_wk")
                ps = pool.tile([P, NH * HD], F32, tag=tg, name=f"pv{st}_{sub}")
                for ko in range(KO):
                    nc.tensor.matmul(
                        ps[:],
                        lhsT=xts[st][:, ko, sub * P:(sub + 1) * P],
                        rhs=wv_sb[:, ko, :],
                        start=(ko == 0),
                        stop=(ko == KO - 1),
                    )
                sc = st * (PSTRIPE // P) + sub
                nc.vector.tensor_copy(
                    v_sb[:, sc, :, 0:HD],
                    ps.rearrange("p (h e) -> p h e", h=NH),
                )

            return [lambda: one(2 * subp, st == 0),
                    lambda: one(2 * subp + 1, False)]

        ySbs = [None] * N_SB

        def out_unit(sb, sub):
            """Output projection for s-rows sb*512+sub*128..+128. Two closures."""
            cell = {}

            def half(nt):
                ps = ps_wk.tile([P, 512], F32, tag="ps_wk", name=f"ops{sb}_{sub}_{nt}")
                ySb = ySbs[sb]
                for cc in range(4):
                    nc.tensor.matmul(
                        ps[:],
                        lhsT=ySb[:, cc, sub * P:(sub + 1) * P],
                        rhs=wout_sb[:, cc, nt * 512:(nt + 1) * 512],
                        start=(cc == 0),
                        stop=(cc == 3),
                    )
                if nt == 0:
                    cell["o_t"] = o_pool.tile([P, 2, 512], F32, tag="osb", name=f"ot{sb}_{sub}")
                o_t = cell["o_t"]
                nc.vector.tensor_copy(o_t[:, nt, :], ps[:])
                if nt == 1:
                    row = (sb * (QSB // P) + sub) * P
                    nc.sync.dma_start(
                        out[row:row + P, :], o_t.rearrange("p a b -> p (a b)")
                    )

            return [lambda: half(0), lambda: half(1)]

        # global filler deque
        fillers = []

        def pop_fillers(n):
            for _ in range(n):
                if fillers:
                    fillers.pop(0)()

        def flush_fillers():
            while fillers:
                fillers.pop(0)()

        def attn_sb(sb, tail_reserve=0):
            ySb = y_pool.tile([P, 4, QSB], BF16, tag="ysb", name=f"ysb{sb}")
            ySbs[sb] = ySb
            nch = 4 * (sb + 1)
            pending = []   # deferred normalize multiplies (previous head-pair)
            for hp in range(NH // 2):
                heads = (2 * hp, 2 * hp + 1)
                rc_k = 4 + hp
                y_pss = [
                    ps_y.tile([P, QSB], F32, tag="ps_y", name=f"yps{i}")
                    for i in range(2)
                ]
                for g in range(nch // 2):
                    c0 = 2 * g
                    # causal trim: chunk c only attends q >= c*128; the
                    # diagonal 128x128 block gets a triangular mask on P^T.
                    qoffs = [P * max(0, c0 + i - 4 * sb) for i in range(2)]
                    s2s = [
                        ps_sc.tile([P, 2, QSB], F32, tag="ps_sc", name=f"s2_{i}")
                        for i in range(2)
                    ]
                    for i in range(2):
                        qo = qoffs[i]
                        for (h, s2) in zip(heads, s2s):
                            bp = (h % 2) * HD
                            c = c0 + i
                            nc.tensor.matmul(
                                s2[:, i, qo:],
                                lhsT=qkT[bp:bp + HD, rc_k, c * P:(c + 1) * P],
                                rhs=qkT[bp:bp + HD, hp, sb * QSB + qo:(sb + 1) * QSB],
                                start=True,
                                stop=True,
                            )
                    # cover the exp latency with deferred PE work
                    if len(fillers) > tail_reserve:
                        pop_fillers(1)
                    if g == 0 and pending:
                        pending.pop(0)()
                    for (h, s2, y_ps) in zip(heads, s2s, y_pss):
                        pt = pt_pool.tile([P, 2, QSB], BF16, tag="pt")
                        ptb = pt[:]
                        # single exp over both chunks; columns below the
                        # later chunk's qoff hold stale psum, never read by AV
                        nc.scalar.activation(
                            pt[:, :, qoffs[0]:], s2[:, :, qoffs[0]:],
                            mybir.ActivationFunctionType.Exp,
                            scale=SCALE,
                        )
                        for i in range(2):
                            c = c0 + i
                            qo = qoffs[i]
                            if c >= 4 * sb:
                                # causal mask at the diagonal block: zero
                                # entries with q < k directly on Pool
                                nc.gpsimd.affine_select(
                                    out=ptb[:, i, qo:qo + P],
                                    in_=ptb[:, i, qo:qo + P],
                                    compare_op=mybir.AluOpType.is_ge,
                                    fill=0.0,
                                    base=0,
                                    channel_multiplier=-1,
                                    pattern=[[1, P]],
                                )
                            nc.tensor.matmul(
                                y_ps[0:HD + 1, qo:],
                                lhsT=v_sb[:, c, h, :],
                                rhs=ptb[:, i, qo:],
                                start=(c == 0),
                                stop=(c == nch - 1),
                            )
                # normalize: DVE copies only -- ACT copies would delay the
                # latency-critical exp chain on the ACT queue; the
                # reciprocal chain runs off base-partition-0 staging
                # (reciprocal_approx_fast reads garbage from nonzero bases).
                tail = tail_reserve and hp == NH // 2 - 1
                ybs = []
                rbs = []
                for j, y_ps in enumerate(y_pss):
                    den = r_pool.tile([1, QSB], F32, tag="den", name=f"den{j}")
                    nc.vector.tensor_copy(den[:], y_ps[HD:HD + 1, :])
                    yb = r_pool.tile([HD, QSB], F32, tag="yb", name=f"yb{j}")
                    nc.vector.tensor_copy(yb[:], y_ps[0:HD, :])
                    r = r_pool.tile([1, QSB], F32, tag="r", name=f"r{j}")
                    nc.vector.reciprocal_approx_fast(r[:], den[:])
                    rb = r_pool.tile([1, QSB], BF16, tag="rb", name=f"rb{j}")
                    nc.vector.tensor_copy(rb[:], r[:])
                    rbs.append(rb)
                    ybs.append(yb)
                # broadcast 1/den on the PE (bf16 outer product, 1 col/cycle)
                # so the final multiplies never wait on the gpsimd sequencer
                if tail:
                    # cover the reciprocal latency with reserved PE work
                    pop_fillers(2)
                    for j, (h, yb) in enumerate(zip(heads, ybs)):
                        bp = (h % 2) * HD
                        rbc_ps = ps_y.tile([P, QSB], F32, tag="ps_y",
                                           name=f"rbcps{j}")
                        nc.tensor.matmul(rbc_ps[0:HD, :], lhsT=ones64[:],
                                         rhs=rbs[j][:], start=True, stop=True)
                        nc.vector.tensor_tensor(
                            ySb[bp:bp + HD, hp, :], yb[:], rbc_ps[0:HD, :],
                            mybir.AluOpType.mult,
                        )
                else:
                    def norm_mults(hp=hp, ybs=ybs, rbs=rbs, heads=heads):
                        for j, (h, yb) in enumerate(zip(heads, ybs)):
                            bp = (h % 2) * HD
                            rbc_ps = ps_wk.tile([P, QSB], F32, tag="ps_wk",
                                                name=f"rbcps{hp}_{j}")
                            nc.tensor.matmul(rbc_ps[0:HD, :], lhsT=ones64[:],
                                             rhs=rbs[j][:], start=True,
                                             stop=True)
                            nc.vector.tensor_tensor(
                                ySb[bp:bp + HD, hp, :], yb[:],
                                rbc_ps[0:HD, :],
                                mybir.AluOpType.mult,
                            )
                    pending.append(norm_mults)
            return pending

        # ---- phase schedule ----
        # P0: stripe 0 full (q0, k0-3, v0-3)
        for rcp in range(4):
            for fn in qk_piece(0, rcp):
                fn()
        for subp in range(2):
            for fn in v_piece(0, subp):
                fn()

        # sb0 ||| stripe1
        load_stripe(1)
        for rcp in range(4):
            fillers += qk_piece(1, rcp)
        for subp in range(2):
            fillers += v_piece(1, subp)
        pend = attn_sb(0)
        flush_fillers()
        for fn in pend:
            fn()

        # sb1 ||| stripe2 + q3 (pulled early) + out0
        load_stripe(2)
        load_stripe(3)
        for rcp in range(4):
            fillers += qk_piece(2, rcp)
        for subp in range(2):
            fillers += v_piece(2, subp)
        for rcp in range(2):              # q rows of stripe 3
            fillers += qk_piece(3, rcp)
        for sub in range(4):
            fillers += out_unit(0, sub)
        pend = attn_sb(1)
        flush_fillers()
        for fn in pend:
            fn()

        # sb2 ||| k3 + out1
        for rcp in range(2, 4):           # k rows of stripe 3
            fillers += qk_piece(3, rcp)
        for sub in range(4):
            fillers += out_unit(1, sub)
        pend = attn_sb(2)
        flush_fillers()
        for fn in pend:
            fn()

        # sb3 ||| v3 + out2; a few closures held back to keep the PE warm
        # through the last normalize so the final out-proj runs at max clock
        for subp in range(2):
            fillers += v_piece(3, subp)
        for sub in range(4):
            fillers += out_unit(2, sub)
        pend = attn_sb(3, tail_reserve=6)
        flush_fillers()
        for fn in pend:
            fn()

        # tail: sb3's out-projection
        for sub in range(4):
            for fn in out_unit(3, sub):
                fn()


_NC_CACHE = None


def _build_program():
    global _NC_CACHE
    if _NC_CACHE is not None:
        return _NC_CACHE
    nc = bacc.Bacc("TRN2", target_bir_lowering=False, debug=False)
    xT = nc.dram_tensor("xT", [D, S], BF16, kind="ExternalInput").ap()
    w_qk = nc.dram_tensor("w_qk", [D, 1024], BF16, kind="ExternalInput").ap()
    w_v = nc.dram_tensor("w_v", [D, 512], BF16, kind="ExternalInput").ap()
    w_out = nc.dram_tensor("w_out", [512, D], BF16, kind="ExternalInput").ap()
    out = nc.dram_tensor("out", [S, D], F32, kind="ExternalOutput").ap()
    with tile.TileContext(nc) as tc:
        _attention_kernel(tc, out, xT, w_qk, w_v, w_out)
    nc.compile()
    _NC_CACHE = nc
    return nc


def make_in_maps(x, W_qkv, W_out):
    import ml_dtypes

    bf16 = ml_dtypes.bfloat16
    x = np.ascontiguousarray(np.asarray(x, dtype=np.float32))
    W_qkv = np.asarray(W_qkv, dtype=np.float32)
    W_out = np.asarray(W_out, dtype=np.float32)
    in_maps = []
    for c in range(8):
        b, g = divmod(c, 2)
        lo = 512 * g
        cols = np.arange(lo, lo + 512)
        in_maps.append({
            "xT": np.ascontiguousarray(x[b].T).astype(bf16),
            "w_qk": np.ascontiguousarray(
                np.concatenate([W_qkv[:, cols], W_qkv[:, D + cols]], axis=1)
            ).astype(bf16),
            "w_v": np.ascontiguousarray(W_qkv[:, 2 * D + cols]).astype(bf16),
            "w_out": np.ascontiguousarray(W_out[cols, :]).astype(bf16),
        })
    return in_maps


def combine_outputs(results):
    # results: list of 8 dicts with "out" [S, D]; core c = 2*b + g
    return np.stack(
        [results[2 * b]["out"] + results[2 * b + 1]["out"] for b in range(B)]
    ).astype(np.float32)


def kernel(x, W_qkv, W_out):
    nc = _build_program()
    in_maps = make_in_maps(x, W_qkv, W_out)
    res = run_bass_kernel_spmd(nc, in_maps, core_ids=list(range(8)))
    return combine_outputs(res.results)


if __name__ == "__main__":
    # smoke test against a local numpy reference
    rng = np.random.default_rng(0)
    x = rng.standard_normal((B, S, D), dtype=np.float32)
    W_qkv = (rng.standard_normal((D, 3 * D)) * 0.02).astype(np.float32)
    W_out = (rng.standard_normal((D, D)) * 0.02).astype(np.float32)
    out = kernel(x, W_qkv, W_out)
    print("out", out.shape, out.dtype, float(np.abs(out).mean()))



# revision 16
# speedup vs baseline: 1.0803x; 1.0803x over previous
"""Causal self-attention (B=4, S=2048, D=1024, H=16, hd=64) on 8 TRN2 NeuronCores.

Sharding: batch 4-way x head-group 2-way. Core c = 2*b + g handles batch b and
heads [8g, 8g+8). Each core computes the QKV projection for its heads, causal
flash-style attention, and a partial output projection; the host sums the two
head-group partials per batch.

v2 schedule: the PE is kept continuously busy (p-state!) by a global deque of
~1.7us "filler" units (projection half-pieces, deferred out-projection units)
popped between attention chunk-pairs to cover exp() latency:
  - stripe 3's q-projection is pulled two phases early so sb3's attention can
    start the moment k chunks land;
  - every superblock's out-projection is deferred into the next superblock;
  - a few fillers are held back to keep the PE warm through the last
    normalize, so the final out-projection runs at max clock;
  - y PSUM->SBUF copies run on the ACT engine, triangle masks on GpSimd,
    keeping DVE off the critical path.
Attention per (head-pair, chunk-pair): S^T = K.Q^T lands P^T via exp() in the
layout the P^T.V matmul wants; a ones-column in V yields softmax denominators
(row 64). Head pairs sit on disjoint PE row halves so their QK matmuls stream
concurrently. No running-max: scores are bounded, exp stays finite in fp32.
"""

import sys

for _p in ("/opt/trn_rl_repo",):
    if _p not in sys.path:
        sys.path.insert(0, _p)

from contextlib import ExitStack

import numpy as np

import concourse.bass as bass
import concourse.mybir as mybir
import concourse.tile as tile
from concourse import bacc
from concourse.bass_utils import run_bass_kernel_spmd

F32 = mybir.dt.float32
BF16 = mybir.dt.bfloat16
P = 128
B, S, D = 4, 2048, 1024
HD = 64          # head dim
NH = 8           # heads per core
KO = D // P      # 8 contraction chunks for the projections
QSB = 512        # q superblock (matmul free dim)
N_SB = S // QSB  # 4
N_SC = S // P    # 16 kv chunks
PSTRIPE = 512    # s-stripe for the projection phase
SCALE = 0.125    # 1/sqrt(64)


def _attention_kernel(tc, out, xT, w_qk, w_v, w_out):
    nc = tc.nc
    with ExitStack() as ctx:
        const_pool = ctx.enter_context(tc.tile_pool(name="const", bufs=1))
        qkT_pool = ctx.enter_context(tc.tile_pool(name="qkT", bufs=1))
        v_pool = ctx.enter_context(tc.tile_pool(name="vsb", bufs=1))
        wqk_pool = ctx.enter_context(tc.tile_pool(name="wqk", bufs=1))
        wv_pool = ctx.enter_context(tc.tile_pool(name="wv", bufs=1))
        wout_pool = ctx.enter_context(tc.tile_pool(name="wout", bufs=1))
        xt_pool = ctx.enter_context(tc.tile_pool(name="xt", bufs=3))
        pt_pool = ctx.enter_context(tc.tile_pool(name="pt", bufs=6))
        y_pool = ctx.enter_context(tc.tile_pool(name="ysb", bufs=2))
        r_pool = ctx.enter_context(tc.tile_pool(name="recip", bufs=6))
        o_pool = ctx.enter_context(tc.tile_pool(name="osb", bufs=3))
        # PSUM: scores 2x2 banks + work(proj/out) 2x1 + y accum 2x1 = 8 banks
        ps_sc = ctx.enter_context(tc.tile_pool(name="ps_sc", bufs=2, space="PSUM"))
        ps_wk = ctx.enter_context(tc.tile_pool(name="ps_wk", bufs=2, space="PSUM"))
        ps_y = ctx.enter_context(tc.tile_pool(name="ps_y", bufs=2, space="PSUM"))

        # Causal-mask bias pair: the diagonal 128x128 score block gets
        # S += L^T @ M via one extra accumulating matmul (128 cols) instead
        # of a post-exp mask op. (L^T M)[k, q] = L[q, k] = -240 where q < k,
        # so exp(0.125 * (s - 240)) ~ 2e-12 kills the non-causal entries.
        maskL = const_pool.tile([P, P], BF16, tag="maskL")
        nc.gpsimd.memset(maskL[:], -240.0)
        nc.gpsimd.affine_select(
            out=maskL[:],
            in_=maskL[:],
            compare_op=mybir.AluOpType.is_ge,
            fill=0.0,
            base=-1,
            channel_multiplier=-1,
            pattern=[[1, P]],
        )
        ident = const_pool.tile([P, P], BF16, tag="ident")
        nc.gpsimd.memset(ident[:], 1.0)
        nc.gpsimd.affine_select(
            out=ident[:],
            in_=ident[:],
            compare_op=mybir.AluOpType.is_equal,
            fill=0.0,
            base=0,
            channel_multiplier=-1,
            pattern=[[1, P]],
        )
        ones64 = const_pool.tile([1, HD], F32, tag="ones64")
        nc.gpsimd.memset(ones64[:], 1.0)

        # p-state warmup: junk matmuls with no DMA dependencies keep the
        # PE busy (and its clock ramping) while the first inputs land.
        warm_w = const_pool.tile([P, P], BF16, tag="warmw")
        nc.gpsimd.memset(warm_w[:], 1.0)
        warm_x = const_pool.tile([P, 512], BF16, tag="warmx")
        nc.gpsimd.memset(warm_x[:], 1.0)
        warm_ps = ps_y.tile([P, 512], F32, tag="ps_y", name="warmps")
        # q^T/k^T store: row-chunk rc<4 holds q rows, rc>=4 holds k rows.
        # Head h lives at partitions 64*(h%2)..+64 of row-chunk h//2 (+4 for k).
        qkT = qkT_pool.tile([P, 8, S], BF16)
        for _w in range(16):
            nc.tensor.matmul(warm_ps[:], lhsT=warm_w[:], rhs=warm_x[:],
                             start=True, stop=True)
        # V store: [s-partition, kv-chunk, head, hd+1]; last col is ones for the
        # softmax denominator.
        v_sb = v_pool.tile([P, N_SC, NH, HD + 1], BF16)
        nc.gpsimd.memset(v_sb[:, :, :, HD], 1.0)

        # stripe-0 x chunks interleave with the weight chunks so the first
        # projection matmul starts after ~2 chunks instead of the full 5 MB
        wqk_sb = wqk_pool.tile([P, KO, 2 * 512], BF16)
        xts = [None] * 4
        xts[0] = xt_pool.tile([P, KO, PSTRIPE], BF16, tag="xt", name="xt0")
        for ko in range(KO):
            for cg in range(2):
                nc.sync.dma_start(
                    wqk_sb[:, ko, cg * 512:(cg + 1) * 512],
                    w_qk[ko * P:(ko + 1) * P, cg * 512:(cg + 1) * 512],
                )
            nc.sync.dma_start(xts[0][:, ko, :], xT[ko * P:(ko + 1) * P, 0:PSTRIPE])
        wv_sb = wv_pool.tile([P, KO, 512], BF16)
        for ko in range(KO):
            nc.sync.dma_start(wv_sb[:, ko, :], w_v[ko * P:(ko + 1) * P, :])
        wout_sb = wout_pool.tile([P, 4, D], BF16)
        for co in range(4):
            nc.sync.dma_start(wout_sb[:, co, :], w_out[co * P:(co + 1) * P, :])

        def load_stripe(st):
            xt = xt_pool.tile([P, KO, PSTRIPE], BF16, tag="xt", name=f"xt{st}")
            for ko in range(KO):
                nc.sync.dma_start(
                    xt[:, ko, :],
                    xT[ko * P:(ko + 1) * P, st * PSTRIPE:(st + 1) * PSTRIPE],
                )
            xts[st] = xt

        # ---- filler units: closures of ~0.9-1.9us of PE work ----

        def qk_piece(st, rcp):
            """q^T/k^T rows for row-chunks 2*rcp..2*rcp+1 of stripe st.
            Four ~0.85us closures (half-contraction granules) so fillers
            never overshoot the per-iteration PE slack."""
            cells = {}

            def half(rc, koh, alt=False):
                if koh == 0:
                    pool, tg = (ps_y, "ps_y") if alt else (ps_wk, "ps_wk")
                    cells[rc] = pool.tile(
                        [P, PSTRIPE], F32, tag=tg, name=f"pqk{st}_{rc}"
                    )
                ps = cells[rc]
                for ko in range(4 * koh, 4 * koh + 4):
                    nc.tensor.matmul(
                        ps[:],
                        lhsT=wqk_sb[:, ko, rc * P:(rc + 1) * P],
                        rhs=xts[st][:, ko, :],
                        start=(ko == 0),
                        stop=(ko == KO - 1),
                    )
                if koh == 1:
                    nc.vector.tensor_copy(
                        qkT[:, rc, st * PSTRIPE:(st + 1) * PSTRIPE], ps[:]
                    )

            return [lambda: half(2 * rcp, 0, st == 0),
                    lambda: half(2 * rcp, 1),
                    lambda: half(2 * rcp + 1, 0),
                    lambda: half(2 * rcp + 1, 1)]

        def v_piece(st, subp):
            """V rows for s-chunks 4*st+2*subp..+1. Four ~0.85us closures."""
            cells = {}

            def half(sub, koh, alt=False):
                if koh == 0:
                    pool, tg = (ps_y, "ps_y") if alt else (ps_wk, "ps_wk")
                    cells[sub] = pool.tile(
                        [P, NH * HD], F32, tag=tg, name=f"pv{st}_{sub}"
                    )
                ps = cells[sub]
                for ko in range(4 * koh, 4 * koh + 4):
                    nc.tensor.matmul(
                        ps[:],
                        lhsT=xts[st][:, ko, sub * P:(sub + 1) * P],
                        rhs=wv_sb[:, ko, :],
                        start=(ko == 0),
                        stop=(ko == KO - 1),
                    )
                if koh == 1:
                    sc = st * (PSTRIPE // P) + sub
                    nc.vector.tensor_copy(
                        v_sb[:, sc, :, 0:HD],
                        ps.rearrange("p (h e) -> p h e", h=NH),
                    )

            return [lambda: half(2 * subp, 0, st == 0),
                    lambda: half(2 * subp, 1),
                    lambda: half(2 * subp + 1, 0),
                    lambda: half(2 * subp + 1, 1)]

        ySbs = [None] * N_SB

        def out_unit(sb, sub):
            """Output projection for s-rows sb*512+sub*128..+128. Two closures."""
            cell = {}

            def half(nt):
                ps = ps_wk.tile([P, 512], F32, tag="ps_wk", name=f"ops{sb}_{sub}_{nt}")
                ySb = ySbs[sb]
                for cc in range(4):
                    nc.tensor.matmul(
                        ps[:],
                        lhsT=ySb[:, cc, sub * P:(sub + 1) * P],
                        rhs=wout_sb[:, cc, nt * 512:(nt + 1) * 512],
                        start=(cc == 0),
                        stop=(cc == 3),
                    )
                if nt == 0:
                    cell["o_t"] = o_pool.tile([P, 2, 512], F32, tag="osb", name=f"ot{sb}_{sub}")
                o_t = cell["o_t"]
                nc.vector.tensor_copy(o_t[:, nt, :], ps[:])
                if nt == 1:
                    row = (sb * (QSB // P) + sub) * P
                    nc.sync.dma_start(
                        out[row:row + P, :], o_t.rearrange("p a b -> p (a b)")
                    )

            return [lambda: half(0), lambda: half(1)]

        # global filler deque
        fillers = []

        def pop_fillers(n):
            for _ in range(n):
                if fillers:
                    fillers.pop(0)()

        def flush_fillers():
            while fillers:
                fillers.pop(0)()

        def attn_sb(sb, tail_reserve=0):
            ySb = y_pool.tile([P, 4, QSB], BF16, tag="ysb", name=f"ysb{sb}")
            ySbs[sb] = ySb
            nch = 4 * (sb + 1)
            pending = []   # deferred normalize multiplies (previous head-pair)
            for hp in range(NH // 2):
                heads = (2 * hp, 2 * hp + 1)
                rc_k = 4 + hp
                y_pss = [
                    ps_y.tile([P, QSB], F32, tag="ps_y", name=f"yps{i}")
                    for i in range(2)
                ]
                for g in range(nch // 2):
                    c0 = 2 * g
                    # causal trim: chunk c only attends q >= c*128; the
                    # diagonal 128x128 block gets a triangular mask on P^T.
                    qoffs = [P * max(0, c0 + i - 4 * sb) for i in range(2)]
                    s2s = [
                        ps_sc.tile([P, 2, QSB], F32, tag="ps_sc", name=f"s2_{i}")
                        for i in range(2)
                    ]
                    for i in range(2):
                        qo = qoffs[i]
                        c = c0 + i
                        diag = c >= 4 * sb
                        for (h, s2) in zip(heads, s2s):
                            bp = (h % 2) * HD
                            nc.tensor.matmul(
                                s2[:, i, qo:],
                                lhsT=qkT[bp:bp + HD, rc_k, c * P:(c + 1) * P],
                                rhs=qkT[bp:bp + HD, hp, sb * QSB + qo:(sb + 1) * QSB],
                                start=True,
                                stop=not diag,
                            )
                        if diag:
                            # fold the causal mask into the score psum: one
                            # 128-col accumulating matmul per head replaces
                            # the post-exp DVE mask (and its latency hop)
                            for s2 in s2s:
                                nc.tensor.matmul(
                                    s2[:, i, qo:qo + P],
                                    lhsT=maskL[:],
                                    rhs=ident[:],
                                    start=False,
                                    stop=True,
                                    skip_group_check=True,
                                )
                    # cover the exp latency with deferred PE work
                    if len(fillers) > tail_reserve:
                        pop_fillers(1)
                    if g == 0 and pending:
                        pending.pop(0)()
                    for (h, s2, y_ps) in zip(heads, s2s, y_pss):
                        pt = pt_pool.tile([P, 2, QSB], BF16, tag="pt")
                        ptb = pt[:]
                        # single exp over both chunks; columns below the
                        # later chunk's qoff hold stale psum, never read by AV
                        nc.scalar.activation(
                            pt[:, :, qoffs[0]:], s2[:, :, qoffs[0]:],
                            mybir.ActivationFunctionType.Exp,
                            scale=SCALE,
                        )
                        for i in range(2):
                            c = c0 + i
                            qo = qoffs[i]
                            nc.tensor.matmul(
                                y_ps[0:HD + 1, qo:],
                                lhsT=v_sb[:, c, h, :],
                                rhs=ptb[:, i, qo:],
                                start=(c == 0),
                                stop=(c == nch - 1),
                            )
                # normalize: DVE copies only -- ACT copies would delay the
                # latency-critical exp chain on the ACT queue; the
                # reciprocal chain runs off base-partition-0 staging
                # (reciprocal_approx_fast reads garbage from nonzero bases).
                tail = tail_reserve and hp == NH // 2 - 1
                ybs = []
                rs = []
                for j, y_ps in enumerate(y_pss):
                    den = r_pool.tile([1, QSB], F32, tag="den", name=f"den{j}")
                    nc.vector.tensor_copy(den[:], y_ps[HD:HD + 1, :])
                    yb = r_pool.tile([HD, QSB], F32, tag="yb", name=f"yb{j}")
                    nc.vector.tensor_copy(yb[:], y_ps[0:HD, :])
                    r = r_pool.tile([1, QSB], F32, tag="r", name=f"r{j}")
                    nc.vector.reciprocal_approx_fast(r[:], den[:])
                    rs.append(r)
                    if not tail:
                        rbc = r_pool.tile([HD, QSB], F32, tag="rbc", name=f"rbc{j}")
                        nc.gpsimd.partition_broadcast(rbc[:], r[:])
                        ybs.append((yb, rbc))
                    else:
                        ybs.append((yb, None))
                if tail:
                    # cover the reciprocal latency with reserved PE work, then
                    # broadcast 1/den on the PE (f32 outer product) so the
                    # final multiplies don't wait on the gpsimd sequencer
                    pop_fillers(2)
                    for j, (h, (yb, _)) in enumerate(zip(heads, ybs)):
                        bp = (h % 2) * HD
                        rbc_ps = ps_y.tile([P, QSB], F32, tag="ps_y",
                                           name=f"rbcps{j}")
                        nc.tensor.matmul(rbc_ps[0:HD, :], lhsT=ones64[:],
                                         rhs=rs[j][:], start=True, stop=True)
                        nc.vector.tensor_tensor(
                            ySb[bp:bp + HD, hp, :], yb[:], rbc_ps[0:HD, :],
                            mybir.AluOpType.mult,
                        )
                else:
                    def norm_mults(hp=hp, ybs=ybs, heads=heads):
                        for (h, (yb, rbc)) in zip(heads, ybs):
                            bp = (h % 2) * HD
                            nc.vector.tensor_tensor(
                                ySb[bp:bp + HD, hp, :], yb[:], rbc[:],
                                mybir.AluOpType.mult,
                            )
                    pending.append(norm_mults)
            return pending

        # ---- phase schedule ----
        # P0: stripe 0 full (q0, k0-3, v0-3)
        for rcp in range(4):
            for fn in qk_piece(0, rcp):
                fn()
        for subp in range(2):
            for fn in v_piece(0, subp):
                fn()

        # sb0 ||| stripe1
        load_stripe(1)
        for rcp in range(4):
            fillers += qk_piece(1, rcp)
        for subp in range(2):
            fillers += v_piece(1, subp)
        pend = attn_sb(0)
        flush_fillers()
        for fn in pend:
            fn()

        # sb1 ||| stripe2 + q3 (pulled early) + out0
        load_stripe(2)
        load_stripe(3)
        for rcp in range(4):
            fillers += qk_piece(2, rcp)
        for subp in range(2):
            fillers += v_piece(2, subp)
        for rcp in range(2):              # q rows of stripe 3
            fillers += qk_piece(3, rcp)
        for sub in range(4):
            fillers += out_unit(0, sub)
        pend = attn_sb(1)
        flush_fillers()
        for fn in pend:
            fn()

        # sb2 ||| k3 + out1
        for rcp in range(2, 4):           # k rows of stripe 3
            fillers += qk_piece(3, rcp)
        for sub in range(4):
            fillers += out_unit(1, sub)
        pend = attn_sb(2)
        flush_fillers()
        for fn in pend:
            fn()

        # sb3 ||| v3 + out2; a few closures held back to keep the PE warm
        # through the last normalize so the final out-proj runs at max clock
        for subp in range(2):
            fillers += v_piece(3, subp)
        for sub in range(4):
            fillers += out_unit(2, sub)
        pend = attn_sb(3, tail_reserve=6)
        flush_fillers()
        for fn in pend:
            fn()

        # tail: sb3's out-projection
        for sub in range(4):
            for fn in out_unit(3, sub):
                fn()


_NC_CACHE = None


def _build_program():
    global _NC_CACHE
    if _NC_CACHE is not None:
        return _NC_CACHE
    nc = bacc.Bacc("TRN2", target_bir_lowering=False, debug=False)
    xT = nc.dram_tensor("xT", [D, S], BF16, kind="ExternalInput").ap()
    w_qk = nc.dram_tensor("w_qk", [D, 1024], BF16, kind="ExternalInput").ap()
    w_v = nc.dram_tensor("w_v", [D, 512], BF16, kind="ExternalInput").ap()
    w_out = nc.dram_tensor("w_out", [512, D], BF16, kind="ExternalInput").ap()
    out = nc.dram_tensor("out", [S, D], F32, kind="ExternalOutput").ap()
    with tile.TileContext(nc) as tc:
        _attention_kernel(tc, out, xT, w_qk, w_v, w_out)
    nc.compile()
    _NC_CACHE = nc
    return nc


def make_in_maps(x, W_qkv, W_out):
    import ml_dtypes

    bf16 = ml_dtypes.bfloat16
    x = np.ascontiguousarray(np.asarray(x, dtype=np.float32))
    W_qkv = np.asarray(W_qkv, dtype=np.float32)
    W_out = np.asarray(W_out, dtype=np.float32)
    in_maps = []
    for c in range(8):
        b, g = divmod(c, 2)
        lo = 512 * g
        cols = np.arange(lo, lo + 512)
        in_maps.append({
            "xT": np.ascontiguousarray(x[b].T).astype(bf16),
            "w_qk": np.ascontiguousarray(
                np.concatenate([W_qkv[:, cols], W_qkv[:, D + cols]], axis=1)
            ).astype(bf16),
            "w_v": np.ascontiguousarray(W_qkv[:, 2 * D + cols]).astype(bf16),
            "w_out": np.ascontiguousarray(W_out[cols, :]).astype(bf16),
        })
    return in_maps


def combine_outputs(results):
    # results: list of 8 dicts with "out" [S, D]; core c = 2*b + g
    return np.stack(
        [results[2 * b]["out"] + results[2 * b + 1]["out"] for b in range(B)]
    ).astype(np.float32)


def kernel(x, W_qkv, W_out):
    nc = _build_program()
    in_maps = make_in_maps(x, W_qkv, W_out)
    res = run_bass_kernel_spmd(nc, in_maps, core_ids=list(range(8)))
    return combine_outputs(res.results)


if __name__ == "__main__":
    # smoke test against a local numpy reference
    rng = np.random.default_rng(0)
    x = rng.standard_normal((B, S, D), dtype=np.float32)
    W_qkv = (rng.standard_normal((D, 3 * D)) * 0.02).astype(np.float32)
    W_out = (rng.standard_normal((D, D)) * 0.02).astype(np.float32)
    out = kernel(x, W_qkv, W_out)
    print("out", out.shape, out.dtype, float(np.abs(out).mean()))

